# revision 12
# baseline (speedup 1.0000x reference)
"""DrugGNN segment-mean + linear embed kernel for 8 Trainium2 NeuronCores.

Strategy (exploits sorted segment_ids):
  - 16384 segments are split into 8 contiguous ranges of 2048 segments, one
    per core; node shards follow via searchsorted, so every segment's nodes
    live wholly on one core (no collectives needed).
  - Each core's 2048 segments form 64 blocks of 32 segments. Host relays
    out each block's nodes into [128, T*65] slabs (nodes on partitions,
    65 = 64 features + a ones column that yields counts), zero-padded to a
    common tile count T, plus per-node relative segment ids (-1 = padding).
  - Device, per 128-node tile: build a [128, 32] one-hot via
    tensor_scalar(is_equal) against an iota, then one accumulating matmul
    onehot.T @ [x|1] -> PSUM[32-seg slice, 65]. Four blocks pack into one
    PSUM [128, 65] via tile_position col-groups.
  - Per 128-segment group: reciprocal(counts) -> scale sums -> PE transpose
    -> GEMM against [weight.T; bias] (bias folded via the ones column)
    -> [128, 128] output rows.
"""
import numpy as np

N_NODES = 2_000_000
IN_CH = 64
OUT_CH = 128
NUM_GRAPHS = 16384
N_CORES = 8
W = 32                      # segments per block (onehot width)
SEGS_PER_CORE = NUM_GRAPHS // N_CORES
NB = SEGS_PER_CORE // W     # blocks per core
NGROUP = NB // 4            # PSUM groups per core (128 segs each)
P = 128                     # nodes per tile / partitions
COLS = IN_CH + 1            # x cols + ones col
XSTRIDE = IN_CH + 1         # per-tile stride in the x slab

TRACE = False               # set True from test harness to capture a profile
LAST_RESULT = None          # BassKernelResults of the last run (for profiling)

_BUILD_CACHE = {}


def _build(T, x_bufs):
    from contextlib import ExitStack
    import concourse.bass as bass
    import concourse.bacc as bacc
    import concourse.tile as tile
    from concourse import mybir

    nc = bacc.Bacc("TRN2", target_bir_lowering=False, debug=False,
                   num_devices=N_CORES)
    xb = nc.dram_tensor("xb", [P, NB * T * XSTRIDE], mybir.dt.bfloat16,
                        kind="ExternalInput").ap()
    srel = nc.dram_tensor("srel", [P, NB * T], mybir.dt.int8,
                          kind="ExternalInput").ap()
    wb = nc.dram_tensor("wb", [COLS, OUT_CH], mybir.dt.float32,
                        kind="ExternalInput").ap()
    ident = nc.dram_tensor("ident", [P, P], mybir.dt.float32,
                           kind="ExternalInput").ap()
    out = nc.dram_tensor("out", [SEGS_PER_CORE, OUT_CH], mybir.dt.float32,
                         kind="ExternalOutput").ap()

    with tile.TileContext(nc) as tc, ExitStack() as ctx:
        singles = ctx.enter_context(tc.tile_pool(name="singles", bufs=1))
        xpool = ctx.enter_context(tc.tile_pool(name="xpool", bufs=x_bufs))
        ohpool = ctx.enter_context(tc.tile_pool(name="ohpool", bufs=6))
        meanpool = ctx.enter_context(tc.tile_pool(name="meanpool", bufs=2))
        sbtpool = ctx.enter_context(tc.tile_pool(name="sbtpool", bufs=2))
        outpool = ctx.enter_context(tc.tile_pool(name="outpool", bufs=2))
        invpool = ctx.enter_context(tc.tile_pool(name="invpool", bufs=2))
        psum_acc = ctx.enter_context(tc.tile_pool(name="psum_acc", bufs=3, space="PSUM"))
        psum_t = ctx.enter_context(tc.tile_pool(name="psum_t", bufs=2, space="PSUM"))
        psum_o = ctx.enter_context(tc.tile_pool(name="psum_o", bufs=2, space="PSUM"))

        srel_sb = singles.tile([P, NB * T], mybir.dt.int8)
        nc.sync.dma_start(srel_sb, srel)
        iota_i = singles.tile([P, W], mybir.dt.int32)
        nc.gpsimd.iota(iota_i, pattern=[[1, W]], base=0, channel_multiplier=0)
        iota_f = singles.tile([P, W], mybir.dt.float32)
        nc.vector.tensor_copy(iota_f, iota_i)
        wb_sb = singles.tile([COLS, OUT_CH], mybir.dt.float32)
        nc.sync.dma_start(wb_sb, wb)
        ident_sb = singles.tile([P, P], mybir.dt.float32)
        nc.sync.dma_start(ident_sb, ident)

        def bcast_iota(t_):
            # [p][t step0][w step1] — reuse the same 32 iota cols per tile
            return bass.AP(tensor=t_.tensor, offset=t_.offset,
                           ap=[t_.ap[0], [0, T], [1, W]])

        def bcast_srel(t_):
            # [p][t step1][w step0] — repeat each tile's scalar across W
            return bass.AP(tensor=t_.tensor, offset=t_.offset,
                           ap=[t_.ap[0], [1, T], [0, W]])

        def epilogue(g, acc):
            # counts -> reciprocal (DVE, tiny); sums*inv (ACT); ones col
            # (GpSimd); transpose+GEMM (PE); copies (ACT); out DMA.
            inv = invpool.tile([P, 1], mybir.dt.float32)
            nc.vector.reciprocal(inv, acc[:, IN_CH:IN_CH + 1])
            means = meanpool.tile([P, COLS], mybir.dt.float32)
            nc.scalar.activation(
                means[:, 0:IN_CH], acc[:, 0:IN_CH],
                mybir.ActivationFunctionType.Copy, bias=0.0, scale=inv)
            nc.gpsimd.memset(means[:, IN_CH:IN_CH + 1], 1.0)
            pt = psum_t.tile([COLS, P], mybir.dt.float32)
            nc.tensor.transpose(pt, means, ident_sb)
            sbt = sbtpool.tile([COLS, P], mybir.dt.float32)
            nc.scalar.copy(sbt, pt)
            po = psum_o.tile([P, OUT_CH], mybir.dt.float32)
            nc.tensor.matmul(po, lhsT=sbt, rhs=wb_sb, start=True, stop=True)
            osb = outpool.tile([P, OUT_CH], mybir.dt.float32)
            nc.scalar.copy(osb, po)
            for q in range(4):
                nc.scalar.dma_start(
                    out[g * P + 32 * q:g * P + 32 * (q + 1), :],
                    osb[32 * q:32 * (q + 1), :])

        GSLAB = 4 * T * XSTRIDE        # one group's x columns per partition
        accs = {}
        for g in range(NGROUP):
            acc = psum_acc.tile([P, COLS], mybir.dt.float32)
            xs = xpool.tile([P, GSLAB], mybir.dt.bfloat16)
            nc.sync.dma_start(xs, xb[:, g * GSLAB:(g + 1) * GSLAB])
            for j in range(4):
                b = 4 * g + j
                oh = ohpool.tile([P, T * W], mybir.dt.bfloat16)
                nc.vector.tensor_tensor(
                    oh, bcast_iota(iota_f),
                    bcast_srel(srel_sb[:, b * T:(b + 1) * T]),
                    mybir.AluOpType.is_equal)
                for t in range(T):
                    c0 = (j * T + t) * XSTRIDE
                    nc.tensor.matmul(
                        acc[W * j:W * (j + 1), :],
                        lhsT=oh[:, t * W:(t + 1) * W],
                        rhs=xs[:, c0:c0 + COLS],
                        start=(t == 0),
                        stop=(t == T - 1),
                        tile_position=(0, W * j))
                if j == 0 and g >= 1:
                    epilogue(g - 1, accs.pop(g - 1))
            accs[g] = acc
        epilogue(NGROUP - 1, accs.pop(NGROUP - 1))
    nc.compile()
    return nc


def _ensure_ntff_hook():
    """Install the antenv.axon_hooks NTFF-profile shim if the image lacks it.

    bass_utils.run_bass_kernel_spmd(trace=True) under axon imports
    antenv.axon_hooks; this container's antenv has no such submodule, so we
    synthesize one and register the ctypes hook against libaxon_pjrt.so.
    """
    import sys
    import types
    try:
        import antenv.axon_hooks  # noqa: F401
        return
    except ImportError:
        pass
    import antenv
    mod = types.ModuleType("antenv.axon_hooks")
    holder = {"h": None}
    mod.set_axon_ntff_profile_hook = lambda h: holder.__setitem__("h", h)
    mod.get_axon_ntff_profile_hook = lambda: holder["h"]
    sys.modules["antenv.axon_hooks"] = mod
    antenv.axon_hooks = mod
    try:
        from trn_agent_boot.trn_boot import _ntff_profile_via_ctypes
        mod.set_axon_ntff_profile_hook(
            _ntff_profile_via_ctypes("/opt/axon/libaxon_pjrt.so"))
    except Exception as e:  # degrade: tracing skipped, run still works
        print(f"ntff hook unavailable: {e}")


def kernel(x, segment_ids, weight, bias, num_graphs):
    global LAST_RESULT
    from concourse import bass_utils
    if TRACE:
        _ensure_ntff_hook()

    x = np.asarray(x, dtype=np.float32)
    seg = np.asarray(segment_ids).astype(np.int64)
    weight = np.asarray(weight, dtype=np.float32)
    bias = np.asarray(bias, dtype=np.float32)
    G = int(num_graphs)
    assert G == NUM_GRAPHS and x.shape == (N_NODES, IN_CH)

    nblocks = N_CORES * NB  # 512 blocks of W segments, globally
    bounds = np.searchsorted(seg, np.arange(0, G + 1, W))  # [nblocks+1]
    cnts = np.diff(bounds)
    T = int(np.max((cnts + P - 1) // P))

    # Blocked relayout: per block, rows -> [T*128, XSTRIDE] bf16 (ones col at
    # IN_CH, one pad col), then tile-major -> node-on-partition [128, T*XSTRIDE].
    import ml_dtypes
    bf16 = ml_dtypes.bfloat16
    big = np.zeros((nblocks, T * P, XSTRIDE), bf16)
    big[:, :, IN_CH] = 1.0
    srel_flat = np.full((nblocks, T * P), -1.0, np.float32)
    rel_all = (seg - (seg // W) * W).astype(np.float32)
    x_bf = x.astype(bf16)
    for i in range(nblocks):
        s, e = bounds[i], bounds[i + 1]
        n = e - s
        big[i, :n, :IN_CH] = x_bf[s:e]
        srel_flat[i, :n] = rel_all[s:e]
    # per-core per-partition contiguous slab: [NB, T, P, X] -> [P, NB*T*X]
    xb_all = np.ascontiguousarray(
        big.reshape(N_CORES, NB, T, P, XSTRIDE).transpose(0, 3, 1, 2, 4)
    ).reshape(N_CORES, P, NB * T * XSTRIDE)
    # [nblocks, T*P] -> per-core [P, NB*T]: core slab [NB, T, P] -> [P, NB*T]
    srel_all = np.ascontiguousarray(
        srel_flat.reshape(N_CORES, NB, T, P).transpose(0, 3, 1, 2)
    ).reshape(N_CORES, P, NB * T).astype(np.int8)
    wb = np.concatenate([weight.T, bias[None]], axis=0).astype(np.float32)
    ident = np.eye(P, dtype=np.float32)

    key = T
    if key not in _BUILD_CACHE:
        _BUILD_CACHE[key] = _build(T, x_bufs=5)
    nc = _BUILD_CACHE[key]

    in_maps = [
        dict(xb=xb_all[c], srel=srel_all[c], wb=wb, ident=ident)
        for c in range(N_CORES)
    ]
    res = bass_utils.run_bass_kernel_spmd(
        nc, in_maps, core_ids=list(range(N_CORES)), trace=TRACE)
    LAST_RESULT = res
    return np.concatenate(
        [res.results[c]["out"] for c in range(N_CORES)], axis=0)


# revision 13
# speedup vs baseline: 1.0664x; 1.0664x over previous
"""DrugGNN segment-mean + linear embed kernel for 8 Trainium2 NeuronCores.

Strategy (exploits sorted segment_ids):
  - 16384 segments are split into 8 contiguous ranges of 2048 segments, one
    per core; node shards follow via searchsorted, so every segment's nodes
    live wholly on one core (no collectives needed).
  - Each core's 2048 segments form 64 blocks of 32 segments. Host relays
    out each block's nodes into [128, T*65] slabs (nodes on partitions,
    65 = 64 features + a ones column that yields counts), zero-padded to a
    common tile count T, plus per-node relative segment ids (-1 = padding).
  - Device, per 128-node tile: build a [128, 32] one-hot via
    tensor_scalar(is_equal) against an iota, then one accumulating matmul
    onehot.T @ [x|1] -> PSUM[32-seg slice, 65]. Four blocks pack into one
    PSUM [128, 65] via tile_position col-groups.
  - Per 128-segment group: reciprocal(counts) -> scale sums -> PE transpose
    -> GEMM against [weight.T; bias] (bias folded via the ones column)
    -> [128, 128] output rows.
"""
import numpy as np

N_NODES = 2_000_000
IN_CH = 64
OUT_CH = 128
NUM_GRAPHS = 16384
N_CORES = 8
W = 32                      # segments per block (onehot width)
SEGS_PER_CORE = NUM_GRAPHS // N_CORES
NB = SEGS_PER_CORE // W     # blocks per core
NGROUP = NB // 4            # PSUM groups per core (128 segs each)
P = 128                     # nodes per tile / partitions
COLS = IN_CH + 1            # x cols + ones col
XSTRIDE = IN_CH + 1         # per-tile stride in the x slab

TRACE = False               # set True from test harness to capture a profile
LAST_RESULT = None          # BassKernelResults of the last run (for profiling)

_BUILD_CACHE = {}


def _build(T, x_bufs):
    from contextlib import ExitStack
    import concourse.bass as bass
    import concourse.bacc as bacc
    import concourse.tile as tile
    from concourse import mybir

    nc = bacc.Bacc("TRN2", target_bir_lowering=False, debug=False,
                   num_devices=N_CORES)
    xb = nc.dram_tensor("xb", [P, NB * T * XSTRIDE], mybir.dt.bfloat16,
                        kind="ExternalInput").ap()
    srel = nc.dram_tensor("srel", [P, NB * T], mybir.dt.int8,
                          kind="ExternalInput").ap()
    wb = nc.dram_tensor("wb", [COLS, OUT_CH], mybir.dt.float32,
                        kind="ExternalInput").ap()
    ident = nc.dram_tensor("ident", [P, P], mybir.dt.float32,
                           kind="ExternalInput").ap()
    out = nc.dram_tensor("out", [SEGS_PER_CORE, OUT_CH], mybir.dt.float32,
                         kind="ExternalOutput").ap()

    with tile.TileContext(nc) as tc, ExitStack() as ctx:
        singles = ctx.enter_context(tc.tile_pool(name="singles", bufs=1))
        xpool = ctx.enter_context(tc.tile_pool(name="xpool", bufs=x_bufs))
        ohpool = ctx.enter_context(tc.tile_pool(name="ohpool", bufs=6))
        meanpool = ctx.enter_context(tc.tile_pool(name="meanpool", bufs=2))
        sbtpool = ctx.enter_context(tc.tile_pool(name="sbtpool", bufs=2))
        outpool = ctx.enter_context(tc.tile_pool(name="outpool", bufs=2))
        invpool = ctx.enter_context(tc.tile_pool(name="invpool", bufs=2))
        psum_acc = ctx.enter_context(tc.tile_pool(name="psum_acc", bufs=3, space="PSUM"))
        psum_t = ctx.enter_context(tc.tile_pool(name="psum_t", bufs=2, space="PSUM"))
        psum_o = ctx.enter_context(tc.tile_pool(name="psum_o", bufs=2, space="PSUM"))

        srel_sb = singles.tile([P, NB * T], mybir.dt.int8)
        nc.sync.dma_start(srel_sb, srel)
        iota_i = singles.tile([P, W], mybir.dt.int32)
        nc.gpsimd.iota(iota_i, pattern=[[1, W]], base=0, channel_multiplier=0)
        iota_f = singles.tile([P, W], mybir.dt.float32)
        nc.vector.tensor_copy(iota_f, iota_i)
        wb_sb = singles.tile([COLS, OUT_CH], mybir.dt.float32)
        nc.sync.dma_start(wb_sb, wb)
        ident_sb = singles.tile([P, P], mybir.dt.float32)
        nc.sync.dma_start(ident_sb, ident)

        def bcast_iota(t_):
            # [p][t step0][w step1] — reuse the same 32 iota cols per tile
            return bass.AP(tensor=t_.tensor, offset=t_.offset,
                           ap=[t_.ap[0], [0, T], [1, W]])

        def bcast_srel(t_):
            # [p][t step1][w step0] — repeat each tile's scalar across W
            return bass.AP(tensor=t_.tensor, offset=t_.offset,
                           ap=[t_.ap[0], [1, T], [0, W]])

        def epilogue(g, acc):
            # counts -> reciprocal (DVE, tiny); sums*inv (ACT); ones col
            # (GpSimd); transpose+GEMM (PE); copies (ACT); out DMA.
            inv = invpool.tile([P, 1], mybir.dt.float32)
            nc.vector.reciprocal(inv, acc[:, IN_CH:IN_CH + 1])
            means = meanpool.tile([P, COLS], mybir.dt.float32)
            nc.scalar.activation(
                means[:, 0:IN_CH], acc[:, 0:IN_CH],
                mybir.ActivationFunctionType.Copy, bias=0.0, scale=inv)
            nc.gpsimd.memset(means[:, IN_CH:IN_CH + 1], 1.0)
            pt = psum_t.tile([COLS, P], mybir.dt.float32)
            nc.tensor.transpose(pt, means, ident_sb)
            sbt = sbtpool.tile([COLS, P], mybir.dt.float32)
            nc.scalar.copy(sbt, pt)
            po = psum_o.tile([P, OUT_CH], mybir.dt.float32)
            nc.tensor.matmul(po, lhsT=sbt, rhs=wb_sb, start=True, stop=True)
            osb = outpool.tile([P, OUT_CH], mybir.dt.float32)
            nc.scalar.copy(osb, po)
            for q in range(4):
                nc.scalar.dma_start(
                    out[g * P + 32 * q:g * P + 32 * (q + 1), :],
                    osb[32 * q:32 * (q + 1), :])

        BSLAB = T * XSTRIDE            # one block's x columns per partition
        accs = {}
        for g in range(NGROUP):
            acc = psum_acc.tile([P, COLS], mybir.dt.float32)
            for j in range(4):
                b = 4 * g + j
                xs = xpool.tile([P, BSLAB], mybir.dt.bfloat16)
                nc.sync.dma_start(xs, xb[:, b * BSLAB:(b + 1) * BSLAB])
                oh = ohpool.tile([P, T * W], mybir.dt.bfloat16)
                nc.vector.tensor_tensor(
                    oh, bcast_iota(iota_f),
                    bcast_srel(srel_sb[:, b * T:(b + 1) * T]),
                    mybir.AluOpType.is_equal)
                for t in range(T):
                    nc.tensor.matmul(
                        acc[W * j:W * (j + 1), :],
                        lhsT=oh[:, t * W:(t + 1) * W],
                        rhs=xs[:, t * XSTRIDE:t * XSTRIDE + COLS],
                        start=(t == 0),
                        stop=(t == T - 1),
                        tile_position=(0, W * j))
                if j == 0 and g >= 1:
                    epilogue(g - 1, accs.pop(g - 1))
            accs[g] = acc
        epilogue(NGROUP - 1, accs.pop(NGROUP - 1))
    nc.compile()
    return nc


def _ensure_ntff_hook():
    """Install the antenv.axon_hooks NTFF-profile shim if the image lacks it.

    bass_utils.run_bass_kernel_spmd(trace=True) under axon imports
    antenv.axon_hooks; this container's antenv has no such submodule, so we
    synthesize one and register the ctypes hook against libaxon_pjrt.so.
    """
    import sys
    import types
    try:
        import antenv.axon_hooks  # noqa: F401
        return
    except ImportError:
        pass
    import antenv
    mod = types.ModuleType("antenv.axon_hooks")
    holder = {"h": None}
    mod.set_axon_ntff_profile_hook = lambda h: holder.__setitem__("h", h)
    mod.get_axon_ntff_profile_hook = lambda: holder["h"]
    sys.modules["antenv.axon_hooks"] = mod
    antenv.axon_hooks = mod
    try:
        from trn_agent_boot.trn_boot import _ntff_profile_via_ctypes
        mod.set_axon_ntff_profile_hook(
            _ntff_profile_via_ctypes("/opt/axon/libaxon_pjrt.so"))
    except Exception as e:  # degrade: tracing skipped, run still works
        print(f"ntff hook unavailable: {e}")


def kernel(x, segment_ids, weight, bias, num_graphs):
    global LAST_RESULT
    from concourse import bass_utils
    if TRACE:
        _ensure_ntff_hook()

    x = np.asarray(x, dtype=np.float32)
    seg = np.asarray(segment_ids).astype(np.int64)
    weight = np.asarray(weight, dtype=np.float32)
    bias = np.asarray(bias, dtype=np.float32)
    G = int(num_graphs)
    assert G == NUM_GRAPHS and x.shape == (N_NODES, IN_CH)

    nblocks = N_CORES * NB  # 512 blocks of W segments, globally
    bounds = np.searchsorted(seg, np.arange(0, G + 1, W))  # [nblocks+1]
    cnts = np.diff(bounds)
    T = int(np.max((cnts + P - 1) // P))

    # Blocked relayout: per block, rows -> [T*128, XSTRIDE] bf16 (ones col at
    # IN_CH, one pad col), then tile-major -> node-on-partition [128, T*XSTRIDE].
    import ml_dtypes
    bf16 = ml_dtypes.bfloat16
    big = np.zeros((nblocks, T * P, XSTRIDE), bf16)
    big[:, :, IN_CH] = 1.0
    srel_flat = np.full((nblocks, T * P), -1.0, np.float32)
    rel_all = (seg - (seg // W) * W).astype(np.float32)
    x_bf = x.astype(bf16)
    for i in range(nblocks):
        s, e = bounds[i], bounds[i + 1]
        n = e - s
        big[i, :n, :IN_CH] = x_bf[s:e]
        srel_flat[i, :n] = rel_all[s:e]
    # per-core per-partition contiguous slab: [NB, T, P, X] -> [P, NB*T*X]
    xb_all = np.ascontiguousarray(
        big.reshape(N_CORES, NB, T, P, XSTRIDE).transpose(0, 3, 1, 2, 4)
    ).reshape(N_CORES, P, NB * T * XSTRIDE)
    # [nblocks, T*P] -> per-core [P, NB*T]: core slab [NB, T, P] -> [P, NB*T]
    srel_all = np.ascontiguousarray(
        srel_flat.reshape(N_CORES, NB, T, P).transpose(0, 3, 1, 2)
    ).reshape(N_CORES, P, NB * T).astype(np.int8)
    wb = np.concatenate([weight.T, bias[None]], axis=0).astype(np.float32)
    ident = np.eye(P, dtype=np.float32)

    key = T
    if key not in _BUILD_CACHE:
        _BUILD_CACHE[key] = _build(T, x_bufs=6)
    nc = _BUILD_CACHE[key]

    in_maps = [
        dict(xb=xb_all[c], srel=srel_all[c], wb=wb, ident=ident)
        for c in range(N_CORES)
    ]
    res = bass_utils.run_bass_kernel_spmd(
        nc, in_maps, core_ids=list(range(N_CORES)), trace=TRACE)
    LAST_RESULT = res
    return np.concatenate(
        [res.results[c]["out"] for c in range(N_CORES)], axis=0)


# revision 14
# speedup vs baseline: 1.0956x; 1.0274x over previous
"""DrugGNN segment-mean + linear embed kernel for 8 Trainium2 NeuronCores.

Strategy (exploits sorted segment_ids):
  - 16384 segments are split into 8 contiguous ranges of 2048 segments, one
    per core; node shards follow via searchsorted, so every segment's nodes
    live wholly on one core (no collectives needed).
  - Each core's 2048 segments form 64 blocks of 32 segments. Host relays
    out each block's nodes into [128, T*65] slabs (nodes on partitions,
    65 = 64 features + a ones column that yields counts), zero-padded to a
    common tile count T, plus per-node relative segment ids (-1 = padding).
  - Device, per 128-node tile: build a [128, 32] one-hot via
    tensor_scalar(is_equal) against an iota, then one accumulating matmul
    onehot.T @ [x|1] -> PSUM[32-seg slice, 65]. Four blocks pack into one
    PSUM [128, 65] via tile_position col-groups.
  - Per 128-segment group: reciprocal(counts) -> scale sums -> PE transpose
    -> GEMM against [weight.T; bias] (bias folded via the ones column)
    -> [128, 128] output rows.
"""
import numpy as np

N_NODES = 2_000_000
IN_CH = 64
OUT_CH = 128
NUM_GRAPHS = 16384
N_CORES = 8
W = 32                      # segments per block (onehot width)
SEGS_PER_CORE = NUM_GRAPHS // N_CORES
NB = SEGS_PER_CORE // W     # blocks per core
NGROUP = NB // 4            # PSUM groups per core (128 segs each)
P = 128                     # nodes per tile / partitions
COLS = IN_CH + 1            # x cols + ones col
XSTRIDE = IN_CH + 1         # per-tile stride in the x slab

TRACE = False               # set True from test harness to capture a profile
LAST_RESULT = None          # BassKernelResults of the last run (for profiling)

_BUILD_CACHE = {}


def _build(T, x_bufs):
    from contextlib import ExitStack
    import concourse.bass as bass
    import concourse.bacc as bacc
    import concourse.tile as tile
    from concourse import mybir

    nc = bacc.Bacc("TRN2", target_bir_lowering=False, debug=False,
                   num_devices=N_CORES)
    xb = nc.dram_tensor("xb", [P, NB * T * XSTRIDE], mybir.dt.bfloat16,
                        kind="ExternalInput").ap()
    srel = nc.dram_tensor("srel", [P, NB * T], mybir.dt.int8,
                          kind="ExternalInput").ap()
    wb = nc.dram_tensor("wb", [COLS, OUT_CH], mybir.dt.float32,
                        kind="ExternalInput").ap()
    ident = nc.dram_tensor("ident", [P, P], mybir.dt.float32,
                           kind="ExternalInput").ap()
    out = nc.dram_tensor("out", [SEGS_PER_CORE, OUT_CH], mybir.dt.float32,
                         kind="ExternalOutput").ap()

    with tile.TileContext(nc) as tc, ExitStack() as ctx:
        singles = ctx.enter_context(tc.tile_pool(name="singles", bufs=1))
        xpool = ctx.enter_context(tc.tile_pool(name="xpool", bufs=x_bufs))
        ohpool = ctx.enter_context(tc.tile_pool(name="ohpool", bufs=6))
        meanpool = ctx.enter_context(tc.tile_pool(name="meanpool", bufs=2))
        sbtpool = ctx.enter_context(tc.tile_pool(name="sbtpool", bufs=2))
        outpool = ctx.enter_context(tc.tile_pool(name="outpool", bufs=2))
        invpool = ctx.enter_context(tc.tile_pool(name="invpool", bufs=2))
        psum_acc = ctx.enter_context(tc.tile_pool(name="psum_acc", bufs=3, space="PSUM"))
        psum_t = ctx.enter_context(tc.tile_pool(name="psum_t", bufs=2, space="PSUM"))
        psum_o = ctx.enter_context(tc.tile_pool(name="psum_o", bufs=2, space="PSUM"))

        srel_sb = singles.tile([P, NB * T], mybir.dt.int8)
        nc.sync.dma_start(srel_sb, srel)
        iota_i = singles.tile([P, W], mybir.dt.int32)
        nc.gpsimd.iota(iota_i, pattern=[[1, W]], base=0, channel_multiplier=0)
        iota_f = singles.tile([P, W], mybir.dt.float32)
        nc.vector.tensor_copy(iota_f, iota_i)
        wb_sb = singles.tile([COLS, OUT_CH], mybir.dt.float32)
        nc.sync.dma_start(wb_sb, wb)
        ident_sb = singles.tile([P, P], mybir.dt.float32)
        nc.sync.dma_start(ident_sb, ident)

        def bcast_iota(t_):
            # [p][t step0][w step1] — reuse the same 32 iota cols per tile
            return bass.AP(tensor=t_.tensor, offset=t_.offset,
                           ap=[t_.ap[0], [0, T], [1, W]])

        def bcast_srel(t_):
            # [p][t step1][w step0] — repeat each tile's scalar across W
            return bass.AP(tensor=t_.tensor, offset=t_.offset,
                           ap=[t_.ap[0], [1, T], [0, W]])

        def epilogue(g, acc):
            # counts -> reciprocal (DVE, tiny); sums*inv (ACT); ones col
            # (GpSimd); transpose+GEMM (PE); copies (ACT); out DMA.
            inv = invpool.tile([P, 1], mybir.dt.float32)
            nc.vector.reciprocal(inv, acc[:, IN_CH:IN_CH + 1])
            means = meanpool.tile([P, COLS], mybir.dt.float32)
            nc.scalar.activation(
                means[:, 0:IN_CH], acc[:, 0:IN_CH],
                mybir.ActivationFunctionType.Copy, bias=0.0, scale=inv)
            nc.gpsimd.memset(means[:, IN_CH:IN_CH + 1], 1.0)
            pt = psum_t.tile([COLS, P], mybir.dt.float32)
            nc.tensor.transpose(pt, means, ident_sb)
            sbt = sbtpool.tile([COLS, P], mybir.dt.float32)
            nc.scalar.copy(sbt, pt)
            po = psum_o.tile([P, OUT_CH], mybir.dt.float32)
            nc.tensor.matmul(po, lhsT=sbt, rhs=wb_sb, start=True, stop=True)
            osb = outpool.tile([P, OUT_CH], mybir.dt.float32)
            nc.scalar.copy(osb, po)
            nc.scalar.dma_start(out[g * P:(g + 1) * P, :], osb)

        BSLAB = T * XSTRIDE            # one block's x columns per partition
        accs = {}
        for g in range(NGROUP):
            acc = psum_acc.tile([P, COLS], mybir.dt.float32)
            for j in range(4):
                b = 4 * g + j
                xs = xpool.tile([P, BSLAB], mybir.dt.bfloat16)
                nc.sync.dma_start(xs, xb[:, b * BSLAB:(b + 1) * BSLAB])
                oh = ohpool.tile([P, T * W], mybir.dt.bfloat16)
                nc.vector.tensor_tensor(
                    oh, bcast_iota(iota_f),
                    bcast_srel(srel_sb[:, b * T:(b + 1) * T]),
                    mybir.AluOpType.is_equal)
                for t in range(T):
                    nc.tensor.matmul(
                        acc[W * j:W * (j + 1), :],
                        lhsT=oh[:, t * W:(t + 1) * W],
                        rhs=xs[:, t * XSTRIDE:t * XSTRIDE + COLS],
                        start=(t == 0),
                        stop=(t == T - 1),
                        tile_position=(0, W * j))
                if j == 0 and g >= 1:
                    epilogue(g - 1, accs.pop(g - 1))
            accs[g] = acc
        epilogue(NGROUP - 1, accs.pop(NGROUP - 1))
    nc.compile()
    return nc


def _ensure_ntff_hook():
    """Install the antenv.axon_hooks NTFF-profile shim if the image lacks it.

    bass_utils.run_bass_kernel_spmd(trace=True) under axon imports
    antenv.axon_hooks; this container's antenv has no such submodule, so we
    synthesize one and register the ctypes hook against libaxon_pjrt.so.
    """
    import sys
    import types
    try:
        import antenv.axon_hooks  # noqa: F401
        return
    except ImportError:
        pass
    import antenv
    mod = types.ModuleType("antenv.axon_hooks")
    holder = {"h": None}
    mod.set_axon_ntff_profile_hook = lambda h: holder.__setitem__("h", h)
    mod.get_axon_ntff_profile_hook = lambda: holder["h"]
    sys.modules["antenv.axon_hooks"] = mod
    antenv.axon_hooks = mod
    try:
        from trn_agent_boot.trn_boot import _ntff_profile_via_ctypes
        mod.set_axon_ntff_profile_hook(
            _ntff_profile_via_ctypes("/opt/axon/libaxon_pjrt.so"))
    except Exception as e:  # degrade: tracing skipped, run still works
        print(f"ntff hook unavailable: {e}")


def kernel(x, segment_ids, weight, bias, num_graphs):
    global LAST_RESULT
    from concourse import bass_utils
    if TRACE:
        _ensure_ntff_hook()

    x = np.asarray(x, dtype=np.float32)
    seg = np.asarray(segment_ids).astype(np.int64)
    weight = np.asarray(weight, dtype=np.float32)
    bias = np.asarray(bias, dtype=np.float32)
    G = int(num_graphs)
    assert G == NUM_GRAPHS and x.shape == (N_NODES, IN_CH)

    nblocks = N_CORES * NB  # 512 blocks of W segments, globally
    bounds = np.searchsorted(seg, np.arange(0, G + 1, W))  # [nblocks+1]
    cnts = np.diff(bounds)
    T = int(np.max((cnts + P - 1) // P))

    # Blocked relayout: per block, rows -> [T*128, XSTRIDE] bf16 (ones col at
    # IN_CH, one pad col), then tile-major -> node-on-partition [128, T*XSTRIDE].
    import ml_dtypes
    bf16 = ml_dtypes.bfloat16
    big = np.zeros((nblocks, T * P, XSTRIDE), bf16)
    big[:, :, IN_CH] = 1.0
    srel_flat = np.full((nblocks, T * P), -1.0, np.float32)
    rel_all = (seg - (seg // W) * W).astype(np.float32)
    x_bf = x.astype(bf16)
    for i in range(nblocks):
        s, e = bounds[i], bounds[i + 1]
        n = e - s
        big[i, :n, :IN_CH] = x_bf[s:e]
        srel_flat[i, :n] = rel_all[s:e]
    # per-core per-partition contiguous slab: [NB, T, P, X] -> [P, NB*T*X]
    xb_all = np.ascontiguousarray(
        big.reshape(N_CORES, NB, T, P, XSTRIDE).transpose(0, 3, 1, 2, 4)
    ).reshape(N_CORES, P, NB * T * XSTRIDE)
    # [nblocks, T*P] -> per-core [P, NB*T]: core slab [NB, T, P] -> [P, NB*T]
    srel_all = np.ascontiguousarray(
        srel_flat.reshape(N_CORES, NB, T, P).transpose(0, 3, 1, 2)
    ).reshape(N_CORES, P, NB * T).astype(np.int8)
    wb = np.concatenate([weight.T, bias[None]], axis=0).astype(np.float32)
    ident = np.eye(P, dtype=np.float32)

    key = T
    if key not in _BUILD_CACHE:
        _BUILD_CACHE[key] = _build(T, x_bufs=6)
    nc = _BUILD_CACHE[key]

    in_maps = [
        dict(xb=xb_all[c], srel=srel_all[c], wb=wb, ident=ident)
        for c in range(N_CORES)
    ]
    res = bass_utils.run_bass_kernel_spmd(
        nc, in_maps, core_ids=list(range(N_CORES)), trace=TRACE)
    LAST_RESULT = res
    return np.concatenate(
        [res.results[c]["out"] for c in range(N_CORES)], axis=0)
